# revision 34
# baseline (speedup 1.0000x reference)
"""CAM-module (complex channel-attention) Bass kernel for Trainium2.

Problem: x (2, 8, 512, 4, 32, 32) fp32 -> out same shape.
  qr, qi = x[0].reshape(B,C,N), x[1].reshape(B,C,N)   C=512, N=4096
  er = qr qr^T - qi qi^T ; ei = qr qi^T + qi qr^T     (B, C, C)
  F  = (rowmax(er)-er)^2 + (rowmax(ei)-ei)^2
  att = softmax_row(F)                                 (ultra-sharp)
  out = stack(g*att@qr + qr_in, g*att@qi + qi_in)

Sharding: data-parallel over batch B across 8 NeuronCores (core b = batch b).

Numerics: the softmax is near-one-hot with top-2 F-gaps as small as ~18, so
the Gram phase needs fp32-quality precision; single-dtype bf16/tf32 matmuls
flip argmax rows. We use an fp16 hi/lo split (A = f16(q), Bl = f16(q - A))
and compute
  qq'^T ~= A A'^T + A Bl'^T + Bl A'^T   (dropping Bl Bl'^T, err ~1e-5)
at full 1-cycle/row PE rate. Symmetric cross terms are computed once and
symmetrized via PE transposes:
  er = AA_r - AA_i + M12 + M12^T,  M12 = A_r Bl_r^T - A_i Bl_i^T
  ei = S + S^T,                    S   = A_r A_i^T + A_r Bl_i^T + Bl_r A_i^T
Phase 2 (att @ q) runs in fp16 (error ~1e-5 of O(1) outputs).

Schedule notes:
 - input loaded in column-quarters so the first Gram pass interleaves with
   the PE input transposes;
 - symmetrization transpose-adds are emitted incrementally after each
   m-pass evacuation (only the last pass's blocks land in the tail);
 - phase-2 slab DMA is emitted before the softmax tail so it overlaps;
 - discarded keep-warm matmuls are pinned (via explicit deps) into the
   softmax tail and phase-2 slab stalls so no PE-idle gap exceeds the
   ~3.4us HAM window (PE transposes do not count as HAM activity, so
   without them the first matmuls after each stall run at 1.2 GHz).
"""
import sys, os
sys.path.insert(0, '/opt/trn_rl_repo')

import numpy as np
from contextlib import ExitStack

import concourse.bass as bass
import concourse.mybir as mybir
import concourse.tile as tile
from concourse import bacc
from concourse.bass_utils import run_bass_kernel_spmd
from concourse.masks import make_identity
from concourse.tile import add_dep_helper

F32 = mybir.dt.float32
F16 = mybir.dt.float16
AX = mybir.AxisListType
AF = mybir.ActivationFunctionType
OP = mybir.AluOpType

C = 512          # channels
N = 4096         # spatial (4*32*32)
NK = N // 128    # 32 n-chunks (contraction)
MC = C // 128    # 4 c-chunks
NQ = 4           # column quarters for load/transpose pipeline
KQ = NK // NQ    # 8 n-chunks per quarter
NJ = N // 512    # phase-2 column blocks


def _gram_mms(nc, accs, arT, brT, aiT, biT, m, k):
    """The 7 phase-1 matmuls for (m-block, k-chunk), grouped by lhsT."""
    u1, u2, m1, m2, s = accs
    st, sp = (k == 0), (k == NK - 1)
    lo = k * C
    ar_m = arT[:, lo + m * 128: lo + (m + 1) * 128]
    ai_m = aiT[:, lo + m * 128: lo + (m + 1) * 128]
    br_m = brT[:, lo + m * 128: lo + (m + 1) * 128]
    ar_k = arT[:, lo:lo + C]
    ai_k = aiT[:, lo:lo + C]
    br_k = brT[:, lo:lo + C]
    bi_k = biT[:, lo:lo + C]
    # lhsT = ar_m
    nc.tensor.matmul(u1[:], ar_m, ar_k, start=st, stop=sp)
    nc.tensor.matmul(m1[:], ar_m, br_k, start=st, stop=sp)
    nc.tensor.matmul(s[:], ar_m, ai_k, start=st, stop=False)
    nc.tensor.matmul(s[:], ar_m, bi_k, start=False, stop=False)
    # lhsT = ai_m
    nc.tensor.matmul(u2[:], ai_m, ai_k, start=st, stop=sp)
    nc.tensor.matmul(m2[:], ai_m, bi_k, start=st, stop=sp)
    # lhsT = br_m
    nc.tensor.matmul(s[:], br_m, ai_k, start=False, stop=sp)


def build_kernel():
    nc = bacc.Bacc("TRN2", target_bir_lowering=False, debug=False,
                   enable_asserts=False)
    x_dram = nc.dram_tensor("x", (2, C, N), F32, kind="ExternalInput").ap()
    g_dram = nc.dram_tensor("gamma", (1,), F32, kind="ExternalInput").ap()
    y_dram = nc.dram_tensor("y", (2, C, N), F32, kind="ExternalOutput").ap()

    with tile.TileContext(nc) as tc, ExitStack() as ctx:
        const = ctx.enter_context(tc.tile_pool(name="const", bufs=1))
        small = ctx.enter_context(tc.tile_pool(name="small", bufs=10))
        stage = ctx.enter_context(tc.tile_pool(name="stage", bufs=14))
        smx = ctx.enter_context(tc.tile_pool(name="smx", bufs=4))
        sqf = ctx.enter_context(tc.tile_pool(name="sqf", bufs=1))
        sqb = ctx.enter_context(tc.tile_pool(name="sqb", bufs=1))

        ident32 = const.tile([128, 128], F32, tag="id32")
        make_identity(nc, ident32[:])
        ident16 = const.tile([128, 128], F16, tag="id16")
        make_identity(nc, ident16[:])
        ones16 = const.tile([128, 512], F16, tag="ones16")
        nc.gpsimd.memset(ones16[:], 1.0)
        g_bc = const.tile([128, 1], F32, tag="gbc")
        nc.sync.dma_start(g_bc[:], g_dram[None, :].partition_broadcast(128))

        # persistent [512,512] matrices as [128, 4*512] (row-chunk r at cols r*512)
        er_sb = sqf.tile([128, MC * C], F32, tag="er")
        m12_sb = sqf.tile([128, MC * C], F32, tag="m12")
        s_sb = sqf.tile([128, MC * C], F32, tag="s")
        ei_sb = sqf.tile([128, MC * C], F32, tag="ei")
        att_sb = sqb.tile([128, MC * C], F16, tag="att")
        attT_sb = sqb.tile([128, MC * C], F16, tag="attT")

        with tc.tile_pool(name="opsT", bufs=4) as opsT:
            # transposed fp16 hi/lo operands, [128, NK*512]; chunk k at cols k*512
            arT = opsT.tile([128, NK * C], F16, tag="opsT")
            brT = opsT.tile([128, NK * C], F16, tag="opsT")
            aiT = opsT.tile([128, NK * C], F16, tag="opsT")
            biT = opsT.tile([128, NK * C], F16, tag="opsT")
            tens = [(arT, brT), (aiT, biT)]

            with tc.tile_pool(name="acc", bufs=5, space="PSUM") as acc:
                accs = {m: None for m in range(MC)}
                accs[0] = [acc.tile([128, C], F32, tag="acc", name=f"acc0_{i}")
                           for i in range(5)]

                # ------- Phase 0 + first m-pass, interleaved by quarter -------
                with tc.tile_pool(name="tpose", bufs=3, space="PSUM") as tpose:
                    for Q in range(NQ):
                        for ti, (aT, bT) in enumerate(tens):
                            for j in range(MC):     # c-chunk (rows)
                                for kq in range(KQ // 4):  # 4-chunk groups
                                    k0 = Q * KQ + kq * 4
                                    q_t = stage.tile([128, 512], F32, tag="stage")
                                    nc.sync.dma_start(
                                        q_t[:], x_dram[ti, j * 128:(j + 1) * 128,
                                                       k0 * 128:(k0 + 4) * 128])
                                    pt = tpose.tile([128, 512], F32, tag="pt")
                                    for t in range(4):
                                        nc.tensor.transpose(
                                            pt[:, t * 128:(t + 1) * 128],
                                            q_t[:, t * 128:(t + 1) * 128],
                                            ident32[:])
                                    aT_v = aT[:].rearrange("p (k c) -> p k c", c=C)[
                                        :, k0:k0 + 4, j * 128:(j + 1) * 128]
                                    bT_v = bT[:].rearrange("p (k c) -> p k c", c=C)[
                                        :, k0:k0 + 4, j * 128:(j + 1) * 128]
                                    pt_v = pt[:].rearrange("p (t c) -> p t c", c=128)
                                    nc.scalar.copy(aT_v, pt_v)             # f32->f16
                                    nc.vector.tensor_sub(bT_v, pt_v, aT_v)  # lo part
                        # m=0 Gram MMs for this quarter's chunks
                        for k in range(Q * KQ, (Q + 1) * KQ):
                            _gram_mms(nc, accs[0], arT, brT, aiT, biT, 0, k)

                # ------- m-passes + evacuation + incremental symmetrize -------
                with tc.tile_pool(name="symt", bufs=3, space="PSUM") as symt:
                    def evac_and_sym(a):
                        u1, u2, m1, m2, s = accs[a]
                        er_a = er_sb[:, a * C:(a + 1) * C]
                        nc.scalar.copy(er_a, u1[:])
                        nc.vector.tensor_sub(er_a, er_a, u2[:])
                        m12_a = m12_sb[:, a * C:(a + 1) * C]
                        nc.scalar.copy(m12_a, m1[:])
                        nc.vector.tensor_sub(m12_a, m12_a, m2[:])
                        nc.scalar.copy(s_sb[:, a * C:(a + 1) * C], s[:])
                        nc.vector.tensor_add(er_a, er_a, m12_a)  # += M12 row a
                        # blocks (R, Cb) with max(R, Cb) == a are now computable.
                        # Row-a blocks (a, cb<=a) target contiguous er/ei
                        # columns -> batch their transposes into one psum tile
                        # and use a single wide add.
                        w = (a + 1) * 128
                        for nm_, src_sb, dst_sb, add_src in (
                                ("er", m12_sb, er_sb, None),
                                ("ei", s_sb, ei_sb, s_sb)):
                            pt = symt.tile([128, 512], F32, tag="symt",
                                           name=f"symtb_{nm_}_{a}")
                            for cb in range(a + 1):
                                nc.tensor.transpose(
                                    pt[:, cb * 128:(cb + 1) * 128],
                                    src_sb[:, cb * C + a * 128:
                                           cb * C + (a + 1) * 128],
                                    ident32[:])
                            dst = dst_sb[:, a * C: a * C + w]
                            base = (dst if add_src is None
                                    else add_src[:, a * C: a * C + w])
                            nc.vector.tensor_add(dst, base, pt[:, 0:w])
                        # column-a blocks (r, a) for r < a stay as singles
                        for R in range(a):
                            pt = symt.tile([128, 128], F32, tag="symt",
                                           name=f"symt_er_{R}_{a}")
                            nc.tensor.transpose(
                                pt[:],
                                m12_sb[:, a * C + R * 128: a * C + (R + 1) * 128],
                                ident32[:])
                            dst = er_sb[:, R * C + a * 128: R * C + (a + 1) * 128]
                            nc.vector.tensor_add(dst, dst, pt[:])
                            pt2 = symt.tile([128, 128], F32, tag="symt",
                                            name=f"symt_ei_{R}_{a}")
                            nc.tensor.transpose(
                                pt2[:],
                                s_sb[:, a * C + R * 128: a * C + (R + 1) * 128],
                                ident32[:])
                            src = s_sb[:, R * C + a * 128: R * C + (a + 1) * 128]
                            dst = ei_sb[:, R * C + a * 128: R * C + (a + 1) * 128]
                            nc.vector.tensor_add(dst, src, pt2[:])

                    evac_and_sym(0)
                    for m in range(1, MC):
                        accs[m] = [acc.tile([128, C], F32, tag="acc",
                                            name=f"acc{m}_{i}") for i in range(5)]
                        for k in range(NK):
                            _gram_mms(nc, accs[m], arT, brT, aiT, biT, m, k)
                        evac_and_sym(m)

        # ------------- tail (softmax, attT) + Phase 2, overlapped -----------
        with tc.tile_pool(name="slab32", bufs=8) as slab32, \
             tc.tile_pool(name="slab16", bufs=8) as slab16, \
             tc.tile_pool(name="ysb", bufs=12) as ysbp, \
             tc.tile_pool(name="attp", bufs=2, space="PSUM") as attp, \
             tc.tile_pool(name="out", bufs=6, space="PSUM") as outp:

            # emit ALL phase-2 slab DMA up front (overlaps the softmax tail);
            # pool slot rotation (bufs -> column-blocks in flight) throttles
            # DMA. The f16 casts are deliberately NOT emitted here: ACT/DVE
            # execute in program order, and early-emitted casts would queue
            # ahead of the softmax ops, stretching the tail critical path.
            slabs, slabs_h, first_dma = {}, {}, {}

            def emit_slab_load(j):
                for ti in range(2):
                    sl = slab32.tile([128, MC, 512], F32, tag="sl32",
                                     name=f"sl_{j}_{ti}")
                    for d in range(MC):
                        dma = nc.sync.dma_start(
                            sl[:, d, :],
                            x_dram[ti, d * 128:(d + 1) * 128, j * 512:(j + 1) * 512])
                        if ti == 0 and d == 0:
                            first_dma[j] = dma
                    slabs[(j, ti)] = sl

            # prefetch two column-blocks; the rest are emitted inside the
            # compute loop so load and store DMAs interleave in queue order
            for j in range(3):
                emit_slab_load(j)

            # ---------------- softmax over squared magnitude ----------------
            for m in range(MC):
                er_m = er_sb[:, m * C:(m + 1) * C]
                ei_m = ei_sb[:, m * C:(m + 1) * C]
                nmx_r = small.tile([128, 1], F32, tag="small")
                nmx_i = small.tile([128, 1], F32, tag="small")
                nc.vector.reduce_max(nmx_r[:], er_m, axis=AX.X, negate=True)
                nc.vector.reduce_max(nmx_i[:], ei_m, axis=AX.X, negate=True)
                sq1 = smx.tile([128, C], F32, tag="smx")
                nc.scalar.activation(sq1[:], er_m, AF.Square, bias=nmx_r[:, 0:1])
                sq2 = smx.tile([128, C], F32, tag="smx")
                nc.scalar.activation(sq2[:], ei_m, AF.Square, bias=nmx_i[:, 0:1])
                fadd = nc.vector.tensor_add(sq1[:], sq1[:], sq2[:])  # F
                # HAM keep-warm: discarded matmul pinned behind F via an
                # explicit dep so it fires mid-tail. PE-transposes don't count
                # as PE-busy for HAM; without this the first phase-2 matmuls
                # run throttled at 1.2 GHz.
                warm = outp.tile([128, 512], F32, tag="out", name=f"warm_{m}")
                wmm = nc.tensor.matmul(warm[:], ident16[:], ones16[:],
                                       start=True, stop=True)
                add_dep_helper(wmm.ins, fadd.ins, sync=True,
                               reason="HAM keep-warm spacing")
                nfm = small.tile([128, 1], F32, tag="small")
                nc.vector.reduce_max(nfm[:], sq1[:], axis=AX.X, negate=True)
                rsum = small.tile([128, 1], F32, tag="small")
                nc.scalar.activation(sq2[:], sq1[:], AF.Exp,
                                     bias=nfm[:, 0:1], accum_out=rsum[:, 0:1])
                rinv = small.tile([128, 1], F32, tag="small")
                nc.vector.reciprocal(rinv[:], rsum[:])
                nc.vector.tensor_scalar_mul(
                    att_sb[:, m * C:(m + 1) * C], sq2[:], rinv[:, 0:1])
                # attT for this m right away (m-major blocks: d at cols d*128)
                pt = attp.tile([128, C], F16, tag="attt")
                for d in range(MC):
                    nc.tensor.transpose(
                        pt[:, d * 128:(d + 1) * 128],
                        att_sb[:, m * C + d * 128: m * C + (d + 1) * 128],
                        ident16[:])
                nc.scalar.copy(attT_sb[:, m * C:(m + 1) * C], pt[:])

            # ---------------- Phase 2 compute: gamma*(att@q) + x ------------
            for j in range(NJ):
                if j + 3 < NJ:
                    emit_slab_load(j + 3)
                if j >= 2:
                    # HAM keep-warm across any slab-DMA stall at this boundary
                    warm = outp.tile([128, 512], F32, tag="out",
                                     name=f"warmj_{j}")
                    wmm = nc.tensor.matmul(warm[:], ident16[:], ones16[:],
                                           start=True, stop=True)
                    add_dep_helper(wmm.ins, first_dma[j].ins, sync=True,
                                   reason="HAM keep-warm phase-2")
                for ti in range(2):
                    sh = slab16.tile([128, MC, 512], F16, tag="sl16",
                                     name=f"sh_{j}_{ti}")
                    for d in range(MC):
                        if ti == 0:
                            nc.scalar.copy(sh[:, d, :], slabs[(j, ti)][:, d, :])
                        else:
                            nc.vector.tensor_copy(sh[:, d, :], slabs[(j, ti)][:, d, :])
                    slabs_h[(j, ti)] = sh
                    for m in range(MC):
                        ops = outp.tile([128, 512], F32, tag="out")
                        for d in range(MC):
                            nc.tensor.matmul(
                                ops[:],
                                attT_sb[:, m * C + d * 128: m * C + (d + 1) * 128],
                                slabs_h[(j, ti)][:, d, :],
                                start=(d == 0), stop=(d == MC - 1))
                        y_t = ysbp.tile([128, 512], F32, tag="ysb")
                        nc.vector.scalar_tensor_tensor(
                            y_t[:], ops[:], g_bc[:, 0:1], slabs[(j, ti)][:, m, :],
                            op0=OP.mult, op1=OP.add)
                        nc.sync.dma_start(
                            y_dram[ti, m * 128:(m + 1) * 128, j * 512:(j + 1) * 512],
                            y_t[:])

    nc.compile()
    return nc


_NC_CACHE = None


def kernel(x: np.ndarray, gamma: np.ndarray) -> np.ndarray:
    global _NC_CACHE
    if _NC_CACHE is None:
        _NC_CACHE = build_kernel()
    nc = _NC_CACHE
    B = x.shape[1]
    x = np.ascontiguousarray(x, dtype=np.float32)
    in_maps = [{"x": np.ascontiguousarray(x[:, b]).reshape(2, C, N),
                "gamma": np.ascontiguousarray(gamma, dtype=np.float32)}
               for b in range(B)]
    res = run_bass_kernel_spmd(nc, in_maps, core_ids=list(range(B)))
    y = np.stack([res.results[b]["y"] for b in range(B)], axis=1)
    return y.reshape(x.shape)
